# revision 17
# baseline (speedup 1.0000x reference)
"""Transformer encoder layer (LN -> MHA -> residual -> LN -> MLP -> residual)
on 8 Trainium2 NeuronCores.

Sharding: token-parallel over the 4096 (batch*seq) tokens, 512 query-tokens
per core; the 4 cores sharing a batch each redundantly compute the full
2048-token K/V for that batch, so no collectives are needed.

v3: all matmul operands bf16 (fast weight load, psum fp32), K/V weights
cached in SBUF, exp widened to 1024 via two-bank psum tiles, LN entirely on
the vector engine (pow(var+eps,-0.5)) so the scalar activation table stays
on Exp through the whole attention phase, software-pipelined chunk loop: the
next chunk's LayerNorm and K/V projections are issued interleaved with the
current chunk's attention heads so the in-order PE queue always has work
while exp is pending; softmax normalization is folded per-head into the last
chunk.

On-chip layout: activations are kept feature-major ("transposed", [d, token])
so every matmul contracts along the partition dim with weights in natural
[d_in, d_out] layout.  Softmax is computed unnormalized (scores are bounded,
so plain exp is numerically safe and algebraically identical); the denominator
comes for free from a ones-column appended to V.

LayerNorm gains/biases are folded into the following projections on the host
(exact algebra: (g*xhat+b) @ W = xhat @ (diag(g) W) + b @ W).
"""

import numpy as np
import ml_dtypes

import concourse.bass as bass
import concourse.mybir as mybir
from concourse import bacc
from concourse.tile import TileContext
from concourse.bass_utils import run_bass_kernel_spmd
from concourse.masks import make_identity

F32 = mybir.dt.float32
BF16 = mybir.dt.bfloat16
MMDT = BF16
AF = mybir.ActivationFunctionType
ALU = mybir.AluOpType

B, S, D = 2, 2048, 1024
H, HD = 16, 64
DFF = 4 * D
NCORES = 8
QT = 512           # query tokens per core
NCHUNK = S // 512  # kv chunks of 512 tokens
EPS = 1e-5

BF = ml_dtypes.bfloat16


class LNPipe:
    """LayerNorm of one 512-token group, split into issue-schedulable parts.

    stats (DVE-only) -> rows (PE transposes + rank-1 broadcasts) ->
    apply (DMA of x^T + DVE muladd into hT).
    """

    def __init__(self, nc, lnp, psL, psS, bcp, ident, onesb, eps, x_dram, xT_dram, col0, hT):
        # psL is the psK pool: transposes borrow [0:1, 0:128] views of its slots
        self.nc, self.lnp, self.psL, self.psS, self.bcp = nc, lnp, psL, psS, bcp
        self.ident, self.onesb, self.eps = ident, onesb, eps
        self.x_dram, self.xT_dram, self.col0, self.hT = x_dram, xT_dram, col0, hT

    def stats(self):
        nc, lnp = self.nc, self.lnp
        self.mr = []
        for st in range(4):
            xt = lnp.tile([128, D], self.x_dram.dtype, tag="ln_x")
            nc.sync.dma_start(
                out=xt,
                in_=self.x_dram[self.col0 + st * 128:self.col0 + (st + 1) * 128, :])
            stats = lnp.tile([128, 2, 6], F32, tag="ln_st")
            nc.vector.bn_stats(stats[:, 0, :], xt[:, 0:512])
            nc.vector.bn_stats(stats[:, 1, :], xt[:, 512:1024])
            mv = lnp.tile([128, 2], F32, tag="ln_mv")
            nc.vector.bn_aggr(mv, stats)
            mr = lnp.tile([128, 2], F32, tag=f"ln_mr{st}")
            sd = lnp.tile([128, 1], F32, tag="ln_sd")
            nc.scalar.activation(sd, mv[:, 1:2], AF.Sqrt, bias=self.eps[:, 0:1])
            nc.vector.reciprocal(mr[:, 1:2], sd)
            # -mu * rstd
            nc.vector.tensor_scalar(mr[:, 0:1], mv[:, 0:1], mr[:, 1:2], -1.0,
                                    ALU.mult, ALU.mult)
            self.mr.append(mr)

    def rows(self):
        nc, lnp, psL, psS = self.nc, self.lnp, self.psL, self.psS
        mr_row = lnp.tile([1, 512], MMDT, tag="mr_row")
        rs_row = lnp.tile([1, 512], MMDT, tag="rs_row")
        for st in range(4):
            pst = psL.tile([128, 512], F32, tag="psK")
            nc.tensor.transpose(pst[0:1, 0:128], self.mr[st][:, 0:1], self.ident)
            nc.vector.tensor_copy(mr_row[:, st * 128:(st + 1) * 128], pst[0:1, 0:128])
            pst2 = psL.tile([128, 512], F32, tag="psK")
            nc.tensor.transpose(pst2[0:1, 0:128], self.mr[st][:, 1:2], self.ident)
            nc.vector.tensor_copy(rs_row[:, st * 128:(st + 1) * 128], pst2[0:1, 0:128])
        self.mr_bc = self.bcp.tile([128, 512], MMDT, tag="mr")
        nc.gpsimd.partition_broadcast(self.mr_bc, mr_row)
        self.rs_bc = self.bcp.tile([128, 512], MMDT, tag="rs")
        nc.gpsimd.partition_broadcast(self.rs_bc, rs_row)

    def apply(self):
        nc = self.nc
        for dt in range(8):
            nc.sync.dma_start(
                out=self.hT[:, dt, :],
                in_=self.xT_dram[dt * 128:(dt + 1) * 128, self.col0:self.col0 + 512],
            )
            nc.vector.tensor_mul(self.hT[:, dt, :], self.hT[:, dt, :], self.rs_bc)
            nc.vector.tensor_add(self.hT[:, dt, :], self.hT[:, dt, :], self.mr_bc)

    def all(self):
        self.stats()
        self.rows()
        self.apply()


def _build():
    nc = bacc.Bacc(None, target_bir_lowering=False)

    XB = nc.declare_dram_parameter("xb", [S, D], MMDT, isOutput=False)
    XQ = nc.declare_dram_parameter("xq", [QT, D], F32, isOutput=False)
    XBT = nc.declare_dram_parameter("xbt", [D, S], MMDT, isOutput=False)
    XQT = nc.declare_dram_parameter("xqt", [D, QT], MMDT, isOutput=False)
    WQ = nc.declare_dram_parameter("wq", [128, 8, D], MMDT, isOutput=False)
    WK = nc.declare_dram_parameter("wk", [D, D], MMDT, isOutput=False)
    WV = nc.declare_dram_parameter("wv", [D, D], MMDT, isOutput=False)
    WO = nc.declare_dram_parameter("wo", [128, 8, D], MMDT, isOutput=False)
    W1 = nc.declare_dram_parameter("w1", [128, 32, D], MMDT, isOutput=False)
    W2 = nc.declare_dram_parameter("w2", [128, 32, D], MMDT, isOutput=False)
    BQ = nc.declare_dram_parameter("bq", [D], F32, isOutput=False)
    BK = nc.declare_dram_parameter("bk", [D], F32, isOutput=False)
    BV = nc.declare_dram_parameter("bv", [D], F32, isOutput=False)
    BO = nc.declare_dram_parameter("bo", [D], MMDT, isOutput=False)
    B1 = nc.declare_dram_parameter("b1", [DFF], F32, isOutput=False)
    B2 = nc.declare_dram_parameter("b2", [D], MMDT, isOutput=False)
    Y = nc.declare_dram_parameter("y", [QT, D], F32, isOutput=True)

    with TileContext(nc) as tc:
        with (
            tc.tile_pool(name="const", bufs=1) as cpool,
            tc.tile_pool(name="accp", bufs=1) as accp,
        ):
            ident = cpool.tile([128, 128], F32)
            make_identity(nc, ident)
            identb = cpool.tile([128, 128], MMDT)
            make_identity(nc, identb)
            eps = cpool.tile([128, 1], F32)
            nc.vector.memset(eps, EPS)
            onesb = cpool.tile([128, 128], MMDT)  # all-ones, bf16
            nc.vector.memset(onesb, 1.0)
            bqT = cpool.tile([128, 8], F32)
            nc.sync.dma_start(out=bqT, in_=BQ[:].rearrange("(t p) -> p t", p=128))
            bkT = cpool.tile([128, 8], F32)
            nc.sync.dma_start(out=bkT, in_=BK[:].rearrange("(t p) -> p t", p=128))
            b1T = cpool.tile([128, 32], F32)
            nc.sync.dma_start(out=b1T, in_=B1[:].rearrange("(t p) -> p t", p=128))
            bv_bc = cpool.tile([128, D], F32)
            nc.sync.dma_start(out=bv_bc, in_=BV[:].partition_broadcast(128))
            bo_row = cpool.tile([1, D], MMDT)
            nc.sync.dma_start(out=bo_row, in_=BO[:].rearrange("(o n) -> o n", o=1))
            b2_row = cpool.tile([1, D], MMDT)
            nc.sync.dma_start(out=b2_row, in_=B2[:].rearrange("(o n) -> o n", o=1))
            # K/V weights cached in SBUF for the whole kv-chunk loop
            wk_sb = cpool.tile([128, 8, D], MMDT)
            nc.sync.dma_start(out=wk_sb, in_=WK[:].rearrange("(t p) n -> p t n", p=128))
            wv_sb = cpool.tile([128, 8, D], MMDT)
            nc.sync.dma_start(out=wv_sb, in_=WV[:].rearrange("(t p) n -> p t n", p=128))
            wo_sb = cpool.tile([128, 8, D], MMDT)
            nc.sync.dma_start(out=wo_sb, in_=WO[:])

            acc = accp.tile([65, 16, 512], MMDT)  # unnormalized attn^T + denom row
            attn128 = accp.tile([128, 8, 512], MMDT)  # normalized attn^T
            x2 = accp.tile([128, 4, D], F32)  # post-attention residual stream
            xq_sb = accp.tile([128, 4, D], F32)
            nc.sync.dma_start(out=xq_sb, in_=XQ[:].rearrange("(t p) n -> p t n", p=128))

            # ---- projections + attention, streamed + software-pipelined ----
            with (
                tc.tile_pool(name="qp", bufs=1) as qp,
                tc.tile_pool(name="lnp", bufs=2) as lnp,
                tc.tile_pool(name="hTp", bufs=2) as hTp,
                tc.tile_pool(name="ktp", bufs=2) as ktp,
                tc.tile_pool(name="vp", bufs=2) as vp,
                tc.tile_pool(name="wsm", bufs=3) as wsm,
                tc.tile_pool(name="bcp", bufs=2) as bcp,
                tc.tile_pool(name="pp", bufs=6) as ppl,
                tc.tile_pool(name="nrm", bufs=2) as nrm,
                tc.tile_pool(name="psK", bufs=2, space="PSUM") as psK,
                tc.tile_pool(name="psS", bufs=4, space="PSUM") as psS,
                tc.tile_pool(name="psA", bufs=2, space="PSUM") as psA,
            ):
                Q_sb = qp.tile([128, 8, 512], MMDT)  # Q^T [hd, q]

                # LN of the core's own 512 query tokens + Q projection
                hqT = hTp.tile([128, 8, 512], MMDT, tag="hT")
                LNPipe(nc, lnp, psK, psS, bcp, ident, onesb, eps,
                       XQ, XQT, 0, hqT).all()
                for ht in range(8):
                    wcol = wsm.tile([128, D], MMDT, tag="w")
                    nc.sync.dma_start(out=wcol, in_=WQ[:, ht, :])
                    psq = psK.tile([128, 512], F32, tag="psK")
                    for dt in range(8):
                        nc.tensor.matmul(
                            psq, wcol[:, dt * 128:(dt + 1) * 128], hqT[:, dt, :],
                            start=(dt == 0), stop=(dt == 7),
                        )
                    nc.vector.tensor_scalar_add(Q_sb[:, ht, :], psq, bqT[:, ht:ht + 1])

                # per-chunk state
                hT_t = [None] * NCHUNK
                KT_t = [None] * NCHUNK
                V_t = [None] * NCHUNK

                def proj_group(kc, g):
                    """Projection group g (0-7: K head-cols, 8-15: V (hc,st))."""
                    hT, KT, V = hT_t[kc], KT_t[kc], V_t[kc]
                    if g < 8:
                        ht = g
                        psk = psK.tile([128, 512], F32, tag="psK")
                        for dt in range(8):
                            nc.tensor.matmul(
                                psk, wk_sb[:, dt, ht * 128:(ht + 1) * 128], hT[:, dt, :],
                                start=(dt == 0), stop=(dt == 7),
                            )
                        nc.vector.tensor_scalar_add(KT[:, ht, :], psk, bkT[:, ht:ht + 1])
                    else:
                        hc, st = (g - 8) // 4, (g - 8) % 4
                        psv = psK.tile([128, 512], F32, tag="psK")
                        for dt in range(8):
                            nc.tensor.matmul(
                                psv,
                                hT[:, dt, st * 128:(st + 1) * 128],
                                wv_sb[:, dt, hc * 512:(hc + 1) * 512],
                                start=(dt == 0),
                                stop=(dt == 7),
                            )
                        nc.vector.tensor_add(
                            V[:, st, hc * 8:(hc + 1) * 8, 0:64],
                            psv.rearrange("p (h d) -> p h d", h=8),
                            bv_bc[:, hc * 512:(hc + 1) * 512].rearrange(
                                "p (h d) -> p h d", h=8
                            ),
                        )

                def new_chunk(kc):
                    hT_t[kc] = hTp.tile([128, 8, 512], MMDT, tag="hT", name=f"hT{kc}")
                    KT_t[kc] = ktp.tile([128, 8, 512], MMDT, tag="KT", name=f"KT{kc}")
                    V_t[kc] = vp.tile([128, 4, 16, 65], MMDT, tag="V", name=f"V{kc}")
                    nc.vector.memset(V_t[kc][:, :, :, 64:65], 1.0)
                    return LNPipe(nc, lnp, psK, psS, bcp, ident, onesb, eps,
                                  XB, XBT, kc * 512, hT_t[kc])

                def scores_exp(kc, h):
                    ko = (h % 2) * 64
                    kj = h // 2
                    KT = KT_t[kc]
                    p_tiles = []
                    for kt in range(4):
                        pss = psS.tile([128, 512], F32, tag="psS")
                        nc.tensor.matmul(
                            pss,
                            KT[ko:ko + 64, kj, kt * 128:(kt + 1) * 128],
                            Q_sb[ko:ko + 64, kj, :],
                            start=True,
                            stop=True,
                        )
                        P = ppl.tile([128, 512], MMDT, tag="P")
                        nc.scalar.activation(P, pss, AF.Exp, scale=0.125)
                        p_tiles.append(P)
                    return p_tiles

                def av_acc(kc, h, p_tiles):
                    psa = psA.tile([65, 512], F32, tag="psA")
                    V = V_t[kc]
                    for kt in range(4):
                        nc.tensor.matmul(
                            psa, V[:, kt, h, :], p_tiles[kt],
                            start=(kt == 0), stop=(kt == 3),
                        )
                    if kc == 0:
                        with nc.allow_low_precision(reason="attn accum fits bf16"):
                            nc.vector.tensor_copy(acc[:, h, :], psa)
                    else:
                        with nc.allow_low_precision(reason="attn accum fits bf16"):
                            nc.vector.tensor_add(acc[:, h, :], acc[:, h, :], psa)

                def normalize(h):
                    rr = nrm.tile([1, 512], MMDT, tag="rr")
                    with nc.allow_low_precision(reason="softmax denom fits bf16"):
                        nc.vector.reciprocal(rr, acc[64:65, h, :])
                    rbt = nrm.tile([64, 512], MMDT, tag="rbt")
                    nc.gpsimd.partition_broadcast(rbt, rr)
                    ko = (h % 2) * 64
                    nc.vector.tensor_mul(
                        attn128[ko:ko + 64, h // 2, :], acc[0:64, h, :], rbt
                    )

                # chunk 0: LN + projections up front
                ln = new_chunk(0)
                ln.all()
                for g in range(16):
                    proj_group(0, g)

                for kc in range(NCHUNK):
                    last = kc == NCHUNK - 1
                    if not last:
                        nln = new_chunk(kc + 1)
                    p_cur = scores_exp(kc, 0)
                    for h in range(H):
                        p_next = scores_exp(kc, h + 1) if h + 1 < H else None
                        if not last:
                            # stage the next chunk's LN so its DVE/DMA work
                            # overlaps this chunk's exp-bound stretches
                            if h == 0:
                                nln.stats()
                            elif h == 4:
                                nln.rows()
                            elif h == 5:
                                nln.apply()
                            elif h >= 8:  # 2 projection groups per head
                                proj_group(kc + 1, 2 * (h - 8))
                                proj_group(kc + 1, 2 * (h - 8) + 1)
                        av_acc(kc, h, p_cur)
                        if last:
                            normalize(h)
                        p_cur = p_next

            # ---- out-projection + residual ----
            with (
                tc.tile_pool(name="psO", bufs=8, space="PSUM") as psO,
            ):
                po = [psO.tile([128, 512], F32, tag="psO", name=f"po{i}") for i in range(8)]
                for j in range(8):
                    wot = wo_sb[:, j, :]
                    for c in range(2):
                        for qt in range(4):
                            nc.tensor.matmul(
                                po[c * 4 + qt], attn128[:, j, qt * 128:(qt + 1) * 128],
                                wot[:, c * 512:(c + 1) * 512],
                                start=(j == 0), stop=False,
                            )
                for qt in range(4):
                    for c in range(2):
                        # fold bo in via rank-1 matmul, then single residual add
                        nc.tensor.matmul(
                            po[c * 4 + qt], onesb[0:1, 0:128],
                            bo_row[:, c * 512:(c + 1) * 512],
                            start=False, stop=True,
                        )
                        nc.vector.tensor_add(
                            x2[:, qt, c * 512:(c + 1) * 512],
                            po[c * 4 + qt],
                            xq_sb[:, qt, c * 512:(c + 1) * 512],
                        )

            # ---- LN2 + MLP + residual ----
            with (
                tc.tile_pool(name="lnp2", bufs=2) as lnp2,
                tc.tile_pool(name="h2p", bufs=1) as h2p,
                tc.tile_pool(name="gp", bufs=1) as gp,
                tc.tile_pool(name="wfp", bufs=4) as wfp,
                tc.tile_pool(name="w2p", bufs=4) as w2p,
                tc.tile_pool(name="yp", bufs=2) as yp,
            ):
                h2T = h2p.tile([128, 8, 512], MMDT)
                G = gp.tile([128, 32, 512], MMDT)
                with (
                    tc.tile_pool(name="psT2", bufs=2, space="PSUM") as psT2,
                    tc.tile_pool(name="psF", bufs=4, space="PSUM") as psF,
                ):
                    # LN2 on the vector engine + PE transposes
                    for st in range(4):
                        xt = x2[:, st, :]
                        stats = lnp2.tile([128, 2, 6], F32, tag="ln_st")
                        nc.vector.bn_stats(stats[:, 0, :], xt[:, 0:512])
                        nc.vector.bn_stats(stats[:, 1, :], xt[:, 512:1024])
                        mv = lnp2.tile([128, 2], F32, tag="ln_mv")
                        nc.vector.bn_aggr(mv, stats)
                        sd2 = lnp2.tile([128, 1], F32, tag="ln_sd")
                        nc.scalar.activation(sd2, mv[:, 1:2], AF.Sqrt, bias=eps[:, 0:1])
                        rstd = lnp2.tile([128, 1], F32, tag="ln_rs")
                        nc.vector.reciprocal(rstd, sd2)
                        h = lnp2.tile([128, D], MMDT, tag="ln_h")
                        nc.vector.tensor_scalar(h, xt, mv[:, 0:1], rstd[:, 0:1],
                                                ALU.subtract, ALU.mult)
                        for dt in range(8):
                            pst = psT2.tile([128, 128], MMDT, tag="tp")
                            nc.tensor.transpose(pst, h[:, dt * 128:(dt + 1) * 128], identb)
                            nc.vector.tensor_copy(h2T[:, dt, st * 128:(st + 1) * 128], pst)

                    # MLP1: gelu(h2 @ w1 + b1), transposed output [dff, q]
                    for ft in range(32):
                        w1c = wfp.tile([128, D], MMDT, tag="w1")
                        nc.sync.dma_start(out=w1c, in_=W1[:, ft, :])
                        psf = psF.tile([128, 512], F32, tag="psF")
                        for dt in range(8):
                            nc.tensor.matmul(
                                psf, w1c[:, dt * 128:(dt + 1) * 128], h2T[:, dt, :],
                                start=(dt == 0), stop=(dt == 7),
                            )
                        nc.scalar.activation(
                            G[:, ft, :], psf, AF.Gelu, bias=b1T[:, ft:ft + 1]
                        )

                # MLP2: y = G^T @ w2 + b2 + x2
                with tc.tile_pool(name="psY", bufs=8, space="PSUM") as psY:
                    py = [psY.tile([128, 512], F32, tag="psY", name=f"py{i}") for i in range(8)]
                    for ft in range(32):
                        w2t = w2p.tile([128, D], MMDT, tag="w2")
                        nc.sync.dma_start(out=w2t, in_=W2[:, ft, :])
                        for c in range(2):
                            for qt in range(4):
                                nc.tensor.matmul(
                                    py[c * 4 + qt], G[:, ft, qt * 128:(qt + 1) * 128],
                                    w2t[:, c * 512:(c + 1) * 512],
                                    start=(ft == 0), stop=False,
                                )
                    for c in range(2):
                        for qt in range(4):
                            nc.tensor.matmul(
                                py[c * 4 + qt], onesb[0:1, 0:128],
                                b2_row[:, c * 512:(c + 1) * 512],
                                start=False, stop=True,
                            )
                            yt = yp.tile([128, 512], F32, tag="yt2")
                            nc.vector.tensor_add(
                                yt, py[c * 4 + qt], x2[:, qt, c * 512:(c + 1) * 512]
                            )
                            nc.sync.dma_start(
                                out=Y[qt * 128:(qt + 1) * 128, c * 512:(c + 1) * 512],
                                in_=yt,
                            )

    nc.compile()
    return nc


_NC = None


def _get_nc():
    global _NC
    if _NC is None:
        _NC = _build()
    return _NC


def _make_in_maps(x, ln1_g, ln1_b, wq, bq, wk, bk, wv, bv, wo, bo,
                  w1, b1, w2, b2, ln2_g, ln2_b):
    f32 = lambda a: np.ascontiguousarray(np.asarray(a, dtype=np.float32))
    bf = lambda a: np.ascontiguousarray(np.asarray(a, dtype=np.float32).astype(BF))
    x = f32(x)
    ln1_g, ln1_b = f32(ln1_g), f32(ln1_b)
    ln2_g, ln2_b = f32(ln2_g), f32(ln2_b)
    wq, wk, wv, wo = f32(wq), f32(wk), f32(wv), f32(wo)
    w1, w2 = f32(w1), f32(w2)
    bq, bk, bv, bo, b1, b2 = f32(bq), f32(bk), f32(bv), f32(bo), f32(b1), f32(b2)

    # Fold LayerNorm affine params into the following projections (exact).
    # Weight layouts are pre-rearranged so every on-chip DMA reads dense
    # 2KB-per-partition lines:
    #   wq/wo: [p, blk, t*128+j] = w[t*128+p, blk*128+j]
    #   w1:    [p, ft, t*128+j] = w1[t*128+p, ft*128+j]
    #   w2:    [p, ft, n]       = w2[ft*128+p, n]
    def colmajor(w, nblk):  # [1024, nblk*128] -> [128, nblk, 1024]
        return np.ascontiguousarray(
            w.reshape(8, 128, nblk, 128).transpose(1, 2, 0, 3).reshape(128, nblk, 1024))
    def rowmajor(w, nblk):  # [nblk*128, 1024] -> [128, nblk, 1024]
        return np.ascontiguousarray(
            w.reshape(nblk, 128, 1024).transpose(1, 0, 2))
    common = {
        "wq": bf(colmajor(ln1_g[:, None] * wq, 8)),
        "wk": bf(ln1_g[:, None] * wk),
        "wv": bf(ln1_g[:, None] * wv),
        "wo": bf(rowmajor(wo, 8)),
        "w1": bf(colmajor(ln2_g[:, None] * w1, 32)),
        "w2": bf(rowmajor(w2, 32)),
        "bq": f32(bq + ln1_b @ wq),
        "bk": f32(bk + ln1_b @ wk),
        "bv": f32(bv + ln1_b @ wv),
        "bo": bf(bo),
        "b1": f32(b1 + ln2_b @ w1),
        "b2": bf(b2),
    }
    in_maps = []
    for c in range(NCORES):
        b = c // 4
        qoff = (c % 4) * QT
        m = dict(common)
        m["xb"] = bf(x[b])
        m["xq"] = np.ascontiguousarray(x[b, qoff:qoff + QT])
        m["xbt"] = bf(x[b].T)
        m["xqt"] = bf(x[b, qoff:qoff + QT].T)
        in_maps.append(m)
    return in_maps


def kernel(x, ln1_g, ln1_b, wq, bq, wk, bk, wv, bv, wo, bo, w1, b1, w2, b2, ln2_g, ln2_b):
    in_maps = _make_in_maps(x, ln1_g, ln1_b, wq, bq, wk, bk, wv, bv, wo, bo,
                            w1, b1, w2, b2, ln2_g, ln2_b)
    nc = _get_nc()
    res = run_bass_kernel_spmd(nc, in_maps, core_ids=list(range(NCORES)))

    y = np.empty((B, S, D), dtype=np.float32)
    for c in range(NCORES):
        b = c // 4
        qoff = (c % 4) * QT
        y[b, qoff:qoff + QT] = res.results[c]["y"]
    return y


# revision 19
# speedup vs baseline: 1.0417x; 1.0417x over previous
"""Transformer encoder layer (LN -> MHA -> residual -> LN -> MLP -> residual)
on 8 Trainium2 NeuronCores.

Sharding: token-parallel over the 4096 (batch*seq) tokens, 512 query-tokens
per core; the 4 cores sharing a batch each redundantly compute the full
2048-token K/V for that batch, so no collectives are needed.

v3: all matmul operands bf16 (fast weight load, psum fp32), K/V weights
cached in SBUF, exp widened to 1024 via two-bank psum tiles, LN entirely on
the vector engine (pow(var+eps,-0.5)) so the scalar activation table stays
on Exp through the whole attention phase, software-pipelined chunk loop: the
next chunk's LayerNorm and K/V projections are issued interleaved with the
current chunk's attention heads so the in-order PE queue always has work
while exp is pending; softmax normalization is folded per-head into the last
chunk.

On-chip layout: activations are kept feature-major ("transposed", [d, token])
so every matmul contracts along the partition dim with weights in natural
[d_in, d_out] layout.  Softmax is computed unnormalized (scores are bounded,
so plain exp is numerically safe and algebraically identical); the denominator
comes for free from a ones-column appended to V.

LayerNorm gains/biases are folded into the following projections on the host
(exact algebra: (g*xhat+b) @ W = xhat @ (diag(g) W) + b @ W).
"""

import numpy as np
import ml_dtypes

import concourse.bass as bass
import concourse.mybir as mybir
from concourse import bacc
from concourse.tile import TileContext
from concourse.bass_utils import run_bass_kernel_spmd
from concourse.masks import make_identity

F32 = mybir.dt.float32
BF16 = mybir.dt.bfloat16
MMDT = BF16
AF = mybir.ActivationFunctionType
ALU = mybir.AluOpType

B, S, D = 2, 2048, 1024
H, HD = 16, 64
DFF = 4 * D
NCORES = 8
QT = 512           # query tokens per core
NCHUNK = S // 512  # kv chunks of 512 tokens
EPS = 1e-5

BF = ml_dtypes.bfloat16


class LNPipe:
    """LayerNorm of one 512-token group, split into issue-schedulable parts.

    stats (DVE-only) -> rows (PE transposes + rank-1 broadcasts) ->
    apply (DMA of x^T + DVE muladd into hT).
    """

    def __init__(self, nc, lnp, psL, psS, bcp, ident, onesb, eps, x_dram, xT_dram, col0, hT):
        # psL is the psK pool: transposes borrow [0:1, 0:128] views of its slots
        self.nc, self.lnp, self.psL, self.psS, self.bcp = nc, lnp, psL, psS, bcp
        self.ident, self.onesb, self.eps = ident, onesb, eps
        self.x_dram, self.xT_dram, self.col0, self.hT = x_dram, xT_dram, col0, hT

    def stats(self):
        nc, lnp = self.nc, self.lnp
        self.mr = []
        for st in range(4):
            xt = lnp.tile([128, D], self.x_dram.dtype, tag="ln_x")
            nc.sync.dma_start(
                out=xt,
                in_=self.x_dram[self.col0 + st * 128:self.col0 + (st + 1) * 128, :])
            stats = lnp.tile([128, 2, 6], F32, tag="ln_st")
            nc.vector.bn_stats(stats[:, 0, :], xt[:, 0:512])
            nc.vector.bn_stats(stats[:, 1, :], xt[:, 512:1024])
            mv = lnp.tile([128, 2], F32, tag="ln_mv")
            nc.vector.bn_aggr(mv, stats)
            mr = lnp.tile([128, 2], F32, tag=f"ln_mr{st}")
            sd = lnp.tile([128, 1], F32, tag="ln_sd")
            nc.scalar.activation(sd, mv[:, 1:2], AF.Sqrt, bias=self.eps[:, 0:1])
            nc.vector.reciprocal(mr[:, 1:2], sd)
            # -mu * rstd
            nc.vector.tensor_scalar(mr[:, 0:1], mv[:, 0:1], mr[:, 1:2], -1.0,
                                    ALU.mult, ALU.mult)
            self.mr.append(mr)

    def rows(self):
        nc, lnp, psL, psS = self.nc, self.lnp, self.psL, self.psS
        mr_row = lnp.tile([1, 512], MMDT, tag="mr_row")
        rs_row = lnp.tile([1, 512], MMDT, tag="rs_row")
        for st in range(4):
            pst = psL.tile([128, 512], F32, tag="psK")
            nc.tensor.transpose(pst[0:1, 0:128], self.mr[st][:, 0:1], self.ident)
            nc.vector.tensor_copy(mr_row[:, st * 128:(st + 1) * 128], pst[0:1, 0:128])
            pst2 = psL.tile([128, 512], F32, tag="psK")
            nc.tensor.transpose(pst2[0:1, 0:128], self.mr[st][:, 1:2], self.ident)
            nc.vector.tensor_copy(rs_row[:, st * 128:(st + 1) * 128], pst2[0:1, 0:128])
        self.mr_bc = self.bcp.tile([128, 512], MMDT, tag="mr")
        nc.gpsimd.partition_broadcast(self.mr_bc, mr_row)
        self.rs_bc = self.bcp.tile([128, 512], MMDT, tag="rs")
        nc.gpsimd.partition_broadcast(self.rs_bc, rs_row)

    def apply(self):
        nc = self.nc
        for dt in range(8):
            nc.sync.dma_start(
                out=self.hT[:, dt, :],
                in_=self.xT_dram[dt * 128:(dt + 1) * 128, self.col0:self.col0 + 512],
            )
            nc.vector.tensor_mul(self.hT[:, dt, :], self.hT[:, dt, :], self.rs_bc)
            nc.vector.tensor_add(self.hT[:, dt, :], self.hT[:, dt, :], self.mr_bc)

    def all(self):
        self.stats()
        self.rows()
        self.apply()


def _build():
    nc = bacc.Bacc(None, target_bir_lowering=False)

    XB = nc.declare_dram_parameter("xb", [S, D], MMDT, isOutput=False)
    XQ = nc.declare_dram_parameter("xq", [QT, D], F32, isOutput=False)
    XBT = nc.declare_dram_parameter("xbt", [D, S], MMDT, isOutput=False)
    XQT = nc.declare_dram_parameter("xqt", [D, QT], MMDT, isOutput=False)
    WQ = nc.declare_dram_parameter("wq", [128, 8, D], MMDT, isOutput=False)
    WK = nc.declare_dram_parameter("wk", [D, D], MMDT, isOutput=False)
    WV = nc.declare_dram_parameter("wv", [D, D], MMDT, isOutput=False)
    WO = nc.declare_dram_parameter("wo", [128, 8, D], MMDT, isOutput=False)
    W1 = nc.declare_dram_parameter("w1", [128, 32, D], MMDT, isOutput=False)
    W2 = nc.declare_dram_parameter("w2", [128, 32, D], MMDT, isOutput=False)
    BQ = nc.declare_dram_parameter("bq", [D], F32, isOutput=False)
    BK = nc.declare_dram_parameter("bk", [D], F32, isOutput=False)
    BV = nc.declare_dram_parameter("bv", [D], F32, isOutput=False)
    BO = nc.declare_dram_parameter("bo", [D], MMDT, isOutput=False)
    B1 = nc.declare_dram_parameter("b1", [DFF], F32, isOutput=False)
    B2 = nc.declare_dram_parameter("b2", [D], MMDT, isOutput=False)
    Y = nc.declare_dram_parameter("y", [QT, D], F32, isOutput=True)

    with TileContext(nc) as tc:
        with (
            tc.tile_pool(name="const", bufs=1) as cpool,
            tc.tile_pool(name="accp", bufs=1) as accp,
        ):
            ident = cpool.tile([128, 128], F32)
            make_identity(nc, ident)
            identb = cpool.tile([128, 128], MMDT)
            make_identity(nc, identb)
            eps = cpool.tile([128, 1], F32)
            nc.vector.memset(eps, EPS)
            onesb = cpool.tile([128, 128], MMDT)  # all-ones, bf16
            nc.vector.memset(onesb, 1.0)
            bqT = cpool.tile([128, 8], F32)
            nc.sync.dma_start(out=bqT, in_=BQ[:].rearrange("(t p) -> p t", p=128))
            bkT = cpool.tile([128, 8], F32)
            nc.sync.dma_start(out=bkT, in_=BK[:].rearrange("(t p) -> p t", p=128))
            b1T = cpool.tile([128, 32], F32)
            nc.sync.dma_start(out=b1T, in_=B1[:].rearrange("(t p) -> p t", p=128))
            bv_bc = cpool.tile([128, D], F32)
            nc.sync.dma_start(out=bv_bc, in_=BV[:].partition_broadcast(128))
            bo_row = cpool.tile([1, D], MMDT)
            nc.sync.dma_start(out=bo_row, in_=BO[:].rearrange("(o n) -> o n", o=1))
            b2_row = cpool.tile([1, D], MMDT)
            nc.sync.dma_start(out=b2_row, in_=B2[:].rearrange("(o n) -> o n", o=1))
            # K/V weights cached in SBUF for the whole kv-chunk loop
            wk_sb = cpool.tile([128, 8, D], MMDT)
            nc.sync.dma_start(out=wk_sb, in_=WK[:].rearrange("(t p) n -> p t n", p=128))
            wv_sb = cpool.tile([128, 8, D], MMDT)
            nc.sync.dma_start(out=wv_sb, in_=WV[:].rearrange("(t p) n -> p t n", p=128))
            wo_sb = cpool.tile([128, 8, D], MMDT)
            nc.sync.dma_start(out=wo_sb, in_=WO[:])

            acc = accp.tile([65, 16, 512], MMDT)  # unnormalized attn^T + denom row
            attn128 = accp.tile([128, 8, 512], MMDT)  # normalized attn^T
            x2 = accp.tile([128, 4, D], F32)  # post-attention residual stream
            xq_sb = accp.tile([128, 4, D], F32)
            nc.sync.dma_start(out=xq_sb, in_=XQ[:].rearrange("(t p) n -> p t n", p=128))

            # ---- projections + attention, streamed + software-pipelined ----
            with (
                tc.tile_pool(name="qp", bufs=1) as qp,
                tc.tile_pool(name="lnp", bufs=2) as lnp,
                tc.tile_pool(name="hTp", bufs=2) as hTp,
                tc.tile_pool(name="ktp", bufs=2) as ktp,
                tc.tile_pool(name="vp", bufs=2) as vp,
                tc.tile_pool(name="wsm", bufs=3) as wsm,
                tc.tile_pool(name="bcp", bufs=2) as bcp,
                tc.tile_pool(name="pp", bufs=4) as ppl,
                tc.tile_pool(name="nrm", bufs=2) as nrm,
                tc.tile_pool(name="psK", bufs=2, space="PSUM") as psK,
                tc.tile_pool(name="psS", bufs=2, space="PSUM") as psS,
                tc.tile_pool(name="psA", bufs=2, space="PSUM") as psA,
            ):
                Q_sb = qp.tile([128, 8, 512], MMDT)  # Q^T [hd, q]

                # LN of the core's own 512 query tokens + Q projection
                hqT = hTp.tile([128, 8, 512], MMDT, tag="hT")
                LNPipe(nc, lnp, psK, psS, bcp, ident, onesb, eps,
                       XQ, XQT, 0, hqT).all()
                for ht in range(8):
                    wcol = wsm.tile([128, D], MMDT, tag="w")
                    nc.sync.dma_start(out=wcol, in_=WQ[:, ht, :])
                    psq = psK.tile([128, 512], F32, tag="psK")
                    for dt in range(8):
                        nc.tensor.matmul(
                            psq, wcol[:, dt * 128:(dt + 1) * 128], hqT[:, dt, :],
                            start=(dt == 0), stop=(dt == 7),
                        )
                    nc.vector.tensor_scalar_add(Q_sb[:, ht, :], psq, bqT[:, ht:ht + 1])

                # per-chunk state
                hT_t = [None] * NCHUNK
                KT_t = [None] * NCHUNK
                V_t = [None] * NCHUNK

                def proj_group(kc, g):
                    """Projection group g (0-7: K head-cols, 8-15: V (hc,st))."""
                    hT, KT, V = hT_t[kc], KT_t[kc], V_t[kc]
                    if g < 8:
                        ht = g
                        psk = psK.tile([128, 512], F32, tag="psK")
                        for dt in range(8):
                            nc.tensor.matmul(
                                psk, wk_sb[:, dt, ht * 128:(ht + 1) * 128], hT[:, dt, :],
                                start=(dt == 0), stop=(dt == 7),
                            )
                        nc.vector.tensor_scalar_add(KT[:, ht, :], psk, bkT[:, ht:ht + 1])
                    else:
                        hc, st = (g - 8) // 4, (g - 8) % 4
                        psv = psK.tile([128, 512], F32, tag="psK")
                        for dt in range(8):
                            nc.tensor.matmul(
                                psv,
                                hT[:, dt, st * 128:(st + 1) * 128],
                                wv_sb[:, dt, hc * 512:(hc + 1) * 512],
                                start=(dt == 0),
                                stop=(dt == 7),
                            )
                        nc.vector.tensor_add(
                            V[:, st, hc * 8:(hc + 1) * 8, 0:64],
                            psv.rearrange("p (h d) -> p h d", h=8),
                            bv_bc[:, hc * 512:(hc + 1) * 512].rearrange(
                                "p (h d) -> p h d", h=8
                            ),
                        )

                def new_chunk(kc):
                    hT_t[kc] = hTp.tile([128, 8, 512], MMDT, tag="hT", name=f"hT{kc}")
                    KT_t[kc] = ktp.tile([128, 8, 512], MMDT, tag="KT", name=f"KT{kc}")
                    V_t[kc] = vp.tile([128, 4, 16, 65], MMDT, tag="V", name=f"V{kc}")
                    nc.vector.memset(V_t[kc][:, :, :, 64:65], 1.0)
                    return LNPipe(nc, lnp, psK, psS, bcp, ident, onesb, eps,
                                  XB, XBT, kc * 512, hT_t[kc])

                def scores_exp(kc, h):
                    ko = (h % 2) * 64
                    kj = h // 2
                    KT = KT_t[kc]
                    p_tiles = []
                    for half in range(2):
                        pss = psS.tile([128, 2, 512], F32, tag="psS")
                        for j in range(2):
                            kt = half * 2 + j
                            nc.tensor.matmul(
                                pss[:, j, :],
                                KT[ko:ko + 64, kj, kt * 128:(kt + 1) * 128],
                                Q_sb[ko:ko + 64, kj, :],
                                start=True,
                                stop=True,
                            )
                        P = ppl.tile([128, 2, 512], MMDT, tag="P")
                        nc.scalar.activation(P, pss, AF.Exp, scale=0.125)
                        p_tiles.append(P)
                    return p_tiles

                def av_acc(kc, h, p_tiles):
                    psa = psA.tile([65, 512], F32, tag="psA")
                    V = V_t[kc]
                    for kt in range(4):
                        nc.tensor.matmul(
                            psa, V[:, kt, h, :], p_tiles[kt // 2][:, kt % 2, :],
                            start=(kt == 0), stop=(kt == 3),
                        )
                    if kc == 0:
                        with nc.allow_low_precision(reason="attn accum fits bf16"):
                            nc.vector.tensor_copy(acc[:, h, :], psa)
                    else:
                        with nc.allow_low_precision(reason="attn accum fits bf16"):
                            nc.vector.tensor_add(acc[:, h, :], acc[:, h, :], psa)

                def normalize(h):
                    rr = nrm.tile([1, 512], MMDT, tag="rr")
                    with nc.allow_low_precision(reason="softmax denom fits bf16"):
                        nc.vector.reciprocal(rr, acc[64:65, h, :])
                    rbt = nrm.tile([64, 512], MMDT, tag="rbt")
                    nc.gpsimd.partition_broadcast(rbt, rr)
                    ko = (h % 2) * 64
                    nc.vector.tensor_mul(
                        attn128[ko:ko + 64, h // 2, :], acc[0:64, h, :], rbt
                    )

                # chunk 0: LN + projections up front
                ln = new_chunk(0)
                ln.all()
                for g in range(16):
                    proj_group(0, g)

                for kc in range(NCHUNK):
                    last = kc == NCHUNK - 1
                    if not last:
                        nln = new_chunk(kc + 1)
                    p_cur = scores_exp(kc, 0)
                    for h in range(H):
                        p_next = scores_exp(kc, h + 1) if h + 1 < H else None
                        if not last:
                            # stage the next chunk's LN so its DVE/DMA work
                            # overlaps this chunk's exp-bound stretches
                            if h == 0:
                                nln.stats()
                            elif h == 4:
                                nln.rows()
                            elif h == 5:
                                nln.apply()
                            elif h >= 8:  # 2 projection groups per head
                                proj_group(kc + 1, 2 * (h - 8))
                                proj_group(kc + 1, 2 * (h - 8) + 1)
                        av_acc(kc, h, p_cur)
                        if last:
                            normalize(h)
                        p_cur = p_next

            # ---- out-projection + residual ----
            with (
                tc.tile_pool(name="psO", bufs=8, space="PSUM") as psO,
            ):
                po = [psO.tile([128, 512], F32, tag="psO", name=f"po{i}") for i in range(8)]
                for j in range(8):
                    wot = wo_sb[:, j, :]
                    for c in range(2):
                        for qt in range(4):
                            nc.tensor.matmul(
                                po[c * 4 + qt], attn128[:, j, qt * 128:(qt + 1) * 128],
                                wot[:, c * 512:(c + 1) * 512],
                                start=(j == 0), stop=False,
                            )
                for qt in range(4):
                    for c in range(2):
                        # fold bo in via rank-1 matmul, then single residual add
                        nc.tensor.matmul(
                            po[c * 4 + qt], onesb[0:1, 0:128],
                            bo_row[:, c * 512:(c + 1) * 512],
                            start=False, stop=True,
                        )
                        nc.vector.tensor_add(
                            x2[:, qt, c * 512:(c + 1) * 512],
                            po[c * 4 + qt],
                            xq_sb[:, qt, c * 512:(c + 1) * 512],
                        )

            # ---- LN2 + MLP + residual ----
            with (
                tc.tile_pool(name="lnp2", bufs=2) as lnp2,
                tc.tile_pool(name="h2p", bufs=1) as h2p,
                tc.tile_pool(name="gp", bufs=1) as gp,
                tc.tile_pool(name="wfp", bufs=6) as wfp,
                tc.tile_pool(name="w2p", bufs=6) as w2p,
                tc.tile_pool(name="yp", bufs=2) as yp,
            ):
                h2T = h2p.tile([128, 8, 512], MMDT)
                G = gp.tile([128, 32, 512], MMDT)
                with (
                    tc.tile_pool(name="psT2", bufs=2, space="PSUM") as psT2,
                    tc.tile_pool(name="psF", bufs=4, space="PSUM") as psF,
                ):
                    # LN2 on the vector engine + PE transposes
                    for st in range(4):
                        xt = x2[:, st, :]
                        stats = lnp2.tile([128, 2, 6], F32, tag="ln_st")
                        nc.vector.bn_stats(stats[:, 0, :], xt[:, 0:512])
                        nc.vector.bn_stats(stats[:, 1, :], xt[:, 512:1024])
                        mv = lnp2.tile([128, 2], F32, tag="ln_mv")
                        nc.vector.bn_aggr(mv, stats)
                        sd2 = lnp2.tile([128, 1], F32, tag="ln_sd")
                        nc.scalar.activation(sd2, mv[:, 1:2], AF.Sqrt, bias=eps[:, 0:1])
                        rstd = lnp2.tile([128, 1], F32, tag="ln_rs")
                        nc.vector.reciprocal(rstd, sd2)
                        h = lnp2.tile([128, D], MMDT, tag="ln_h")
                        nc.vector.tensor_scalar(h, xt, mv[:, 0:1], rstd[:, 0:1],
                                                ALU.subtract, ALU.mult)
                        for dt in range(8):
                            pst = psT2.tile([128, 128], MMDT, tag="tp")
                            nc.tensor.transpose(pst, h[:, dt * 128:(dt + 1) * 128], identb)
                            nc.vector.tensor_copy(h2T[:, dt, st * 128:(st + 1) * 128], pst)

                    # MLP1: gelu(h2 @ w1 + b1), transposed output [dff, q]
                    for ft in range(32):
                        w1c = wfp.tile([128, D], MMDT, tag="w1")
                        nc.sync.dma_start(out=w1c, in_=W1[:, ft, :])
                        psf = psF.tile([128, 512], F32, tag="psF")
                        for dt in range(8):
                            nc.tensor.matmul(
                                psf, w1c[:, dt * 128:(dt + 1) * 128], h2T[:, dt, :],
                                start=(dt == 0), stop=(dt == 7),
                            )
                        nc.scalar.activation(
                            G[:, ft, :], psf, AF.Gelu, bias=b1T[:, ft:ft + 1]
                        )

                # MLP2: y = G^T @ w2 + b2 + x2
                with tc.tile_pool(name="psY", bufs=8, space="PSUM") as psY:
                    py = [psY.tile([128, 512], F32, tag="psY", name=f"py{i}") for i in range(8)]
                    for ft in range(32):
                        w2t = w2p.tile([128, D], MMDT, tag="w2")
                        nc.sync.dma_start(out=w2t, in_=W2[:, ft, :])
                        for c in range(2):
                            for qt in range(4):
                                nc.tensor.matmul(
                                    py[c * 4 + qt], G[:, ft, qt * 128:(qt + 1) * 128],
                                    w2t[:, c * 512:(c + 1) * 512],
                                    start=(ft == 0), stop=False,
                                )
                    for c in range(2):
                        for qt in range(4):
                            nc.tensor.matmul(
                                py[c * 4 + qt], onesb[0:1, 0:128],
                                b2_row[:, c * 512:(c + 1) * 512],
                                start=False, stop=True,
                            )
                            yt = yp.tile([128, 512], F32, tag="yt2")
                            nc.vector.tensor_add(
                                yt, py[c * 4 + qt], x2[:, qt, c * 512:(c + 1) * 512]
                            )
                            nc.sync.dma_start(
                                out=Y[qt * 128:(qt + 1) * 128, c * 512:(c + 1) * 512],
                                in_=yt,
                            )

    nc.compile()
    return nc


_NC = None


def _get_nc():
    global _NC
    if _NC is None:
        _NC = _build()
    return _NC


def _make_in_maps(x, ln1_g, ln1_b, wq, bq, wk, bk, wv, bv, wo, bo,
                  w1, b1, w2, b2, ln2_g, ln2_b):
    f32 = lambda a: np.ascontiguousarray(np.asarray(a, dtype=np.float32))
    bf = lambda a: np.ascontiguousarray(np.asarray(a, dtype=np.float32).astype(BF))
    x = f32(x)
    ln1_g, ln1_b = f32(ln1_g), f32(ln1_b)
    ln2_g, ln2_b = f32(ln2_g), f32(ln2_b)
    wq, wk, wv, wo = f32(wq), f32(wk), f32(wv), f32(wo)
    w1, w2 = f32(w1), f32(w2)
    bq, bk, bv, bo, b1, b2 = f32(bq), f32(bk), f32(bv), f32(bo), f32(b1), f32(b2)

    # Fold LayerNorm affine params into the following projections (exact).
    # Weight layouts are pre-rearranged so every on-chip DMA reads dense
    # 2KB-per-partition lines:
    #   wq/wo: [p, blk, t*128+j] = w[t*128+p, blk*128+j]
    #   w1:    [p, ft, t*128+j] = w1[t*128+p, ft*128+j]
    #   w2:    [p, ft, n]       = w2[ft*128+p, n]
    def colmajor(w, nblk):  # [1024, nblk*128] -> [128, nblk, 1024]
        return np.ascontiguousarray(
            w.reshape(8, 128, nblk, 128).transpose(1, 2, 0, 3).reshape(128, nblk, 1024))
    def rowmajor(w, nblk):  # [nblk*128, 1024] -> [128, nblk, 1024]
        return np.ascontiguousarray(
            w.reshape(nblk, 128, 1024).transpose(1, 0, 2))
    common = {
        "wq": bf(colmajor(ln1_g[:, None] * wq, 8)),
        "wk": bf(ln1_g[:, None] * wk),
        "wv": bf(ln1_g[:, None] * wv),
        "wo": bf(rowmajor(wo, 8)),
        "w1": bf(colmajor(ln2_g[:, None] * w1, 32)),
        "w2": bf(rowmajor(w2, 32)),
        "bq": f32(bq + ln1_b @ wq),
        "bk": f32(bk + ln1_b @ wk),
        "bv": f32(bv + ln1_b @ wv),
        "bo": bf(bo),
        "b1": f32(b1 + ln2_b @ w1),
        "b2": bf(b2),
    }
    in_maps = []
    for c in range(NCORES):
        b = c // 4
        qoff = (c % 4) * QT
        m = dict(common)
        m["xb"] = bf(x[b])
        m["xq"] = np.ascontiguousarray(x[b, qoff:qoff + QT])
        m["xbt"] = bf(x[b].T)
        m["xqt"] = bf(x[b, qoff:qoff + QT].T)
        in_maps.append(m)
    return in_maps


def kernel(x, ln1_g, ln1_b, wq, bq, wk, bk, wv, bv, wo, bo, w1, b1, w2, b2, ln2_g, ln2_b):
    in_maps = _make_in_maps(x, ln1_g, ln1_b, wq, bq, wk, bk, wv, bv, wo, bo,
                            w1, b1, w2, b2, ln2_g, ln2_b)
    nc = _get_nc()
    res = run_bass_kernel_spmd(nc, in_maps, core_ids=list(range(NCORES)))

    y = np.empty((B, S, D), dtype=np.float32)
    for c in range(NCORES):
        b = c // 4
        qoff = (c % 4) * QT
        y[b, qoff:qoff + QT] = res.results[c]["y"]
    return y


# revision 20
# speedup vs baseline: 1.0670x; 1.0243x over previous
"""Transformer encoder layer (LN -> MHA -> residual -> LN -> MLP -> residual)
on 8 Trainium2 NeuronCores.

Sharding: token-parallel over the 4096 (batch*seq) tokens, 512 query-tokens
per core; the 4 cores sharing a batch each redundantly compute the full
2048-token K/V for that batch, so no collectives are needed.

v3: all matmul operands bf16 (fast weight load, psum fp32), K/V weights
cached in SBUF, exp widened to 1024 via two-bank psum tiles, LN entirely on
the vector engine (pow(var+eps,-0.5)) so the scalar activation table stays
on Exp through the whole attention phase, software-pipelined chunk loop: the
next chunk's LayerNorm and K/V projections are issued interleaved with the
current chunk's attention heads so the in-order PE queue always has work
while exp is pending; softmax normalization is folded per-head into the last
chunk.

On-chip layout: activations are kept feature-major ("transposed", [d, token])
so every matmul contracts along the partition dim with weights in natural
[d_in, d_out] layout.  Softmax is computed unnormalized (scores are bounded,
so plain exp is numerically safe and algebraically identical); the denominator
comes for free from a ones-column appended to V.

LayerNorm gains/biases are folded into the following projections on the host
(exact algebra: (g*xhat+b) @ W = xhat @ (diag(g) W) + b @ W).
"""

import numpy as np
import ml_dtypes

import concourse.bass as bass
import concourse.mybir as mybir
from concourse import bacc
from concourse.tile import TileContext
from concourse.bass_utils import run_bass_kernel_spmd
from concourse.masks import make_identity

F32 = mybir.dt.float32
BF16 = mybir.dt.bfloat16
MMDT = BF16
AF = mybir.ActivationFunctionType
ALU = mybir.AluOpType

B, S, D = 2, 2048, 1024
H, HD = 16, 64
DFF = 4 * D
NCORES = 8
QT = 512           # query tokens per core
NCHUNK = S // 512  # kv chunks of 512 tokens
EPS = 1e-5

BF = ml_dtypes.bfloat16


class LNPipe:
    """LayerNorm of one 512-token group, split into issue-schedulable parts.

    stats (DVE-only) -> rows (PE transposes + rank-1 broadcasts) ->
    apply (DMA of x^T + DVE muladd into hT).
    """

    def __init__(self, nc, lnp, psL, psS, bcp, ident, onesb, eps, x_dram, xT_dram, col0, hT):
        # psL is the psK pool: transposes borrow [0:1, 0:128] views of its slots
        self.nc, self.lnp, self.psL, self.psS, self.bcp = nc, lnp, psL, psS, bcp
        self.ident, self.onesb, self.eps = ident, onesb, eps
        self.x_dram, self.xT_dram, self.col0, self.hT = x_dram, xT_dram, col0, hT

    def stats(self):
        nc, lnp = self.nc, self.lnp
        self.mr = []
        for st in range(4):
            xt = lnp.tile([128, D], self.x_dram.dtype, tag="ln_x")
            nc.sync.dma_start(
                out=xt,
                in_=self.x_dram[self.col0 + st * 128:self.col0 + (st + 1) * 128, :])
            stats = lnp.tile([128, 2, 6], F32, tag="ln_st")
            nc.vector.bn_stats(stats[:, 0, :], xt[:, 0:512])
            nc.vector.bn_stats(stats[:, 1, :], xt[:, 512:1024])
            mv = lnp.tile([128, 2], F32, tag="ln_mv")
            nc.vector.bn_aggr(mv, stats)
            mr = lnp.tile([128, 2], F32, tag=f"ln_mr{st}")
            sd = lnp.tile([128, 1], F32, tag="ln_sd")
            nc.scalar.activation(sd, mv[:, 1:2], AF.Sqrt, bias=self.eps[:, 0:1])
            nc.vector.reciprocal(mr[:, 1:2], sd)
            # -mu * rstd
            nc.vector.tensor_scalar(mr[:, 0:1], mv[:, 0:1], mr[:, 1:2], -1.0,
                                    ALU.mult, ALU.mult)
            self.mr.append(mr)

    def rows(self):
        nc, lnp, psL, psS = self.nc, self.lnp, self.psL, self.psS
        mr_row = lnp.tile([1, 512], MMDT, tag="mr_row")
        rs_row = lnp.tile([1, 512], MMDT, tag="rs_row")
        for st in range(4):
            pst = psL.tile([128, 512], F32, tag="psK")
            nc.tensor.transpose(pst[0:1, 0:128], self.mr[st][:, 0:1], self.ident)
            nc.vector.tensor_copy(mr_row[:, st * 128:(st + 1) * 128], pst[0:1, 0:128])
            pst2 = psL.tile([128, 512], F32, tag="psK")
            nc.tensor.transpose(pst2[0:1, 0:128], self.mr[st][:, 1:2], self.ident)
            nc.vector.tensor_copy(rs_row[:, st * 128:(st + 1) * 128], pst2[0:1, 0:128])
        self.mr_bc = self.bcp.tile([128, 512], MMDT, tag="mr")
        nc.gpsimd.partition_broadcast(self.mr_bc, mr_row)
        self.rs_bc = self.bcp.tile([128, 512], MMDT, tag="rs")
        nc.gpsimd.partition_broadcast(self.rs_bc, rs_row)

    def apply(self):
        nc = self.nc
        for dt in range(8):
            nc.sync.dma_start(
                out=self.hT[:, dt, :],
                in_=self.xT_dram[dt * 128:(dt + 1) * 128, self.col0:self.col0 + 512],
            )
            nc.vector.tensor_mul(self.hT[:, dt, :], self.hT[:, dt, :], self.rs_bc)
            nc.vector.tensor_add(self.hT[:, dt, :], self.hT[:, dt, :], self.mr_bc)

    def all(self):
        self.stats()
        self.rows()
        self.apply()


def _build():
    nc = bacc.Bacc(None, target_bir_lowering=False)

    XB = nc.declare_dram_parameter("xb", [S, D], MMDT, isOutput=False)
    XQ = nc.declare_dram_parameter("xq", [QT, D], F32, isOutput=False)
    XBT = nc.declare_dram_parameter("xbt", [D, S], MMDT, isOutput=False)
    XQT = nc.declare_dram_parameter("xqt", [D, QT], MMDT, isOutput=False)
    WQ = nc.declare_dram_parameter("wq", [128, 8, D], MMDT, isOutput=False)
    WK = nc.declare_dram_parameter("wk", [D, D], MMDT, isOutput=False)
    WV = nc.declare_dram_parameter("wv", [D, D], MMDT, isOutput=False)
    WO = nc.declare_dram_parameter("wo", [128, 8, D], MMDT, isOutput=False)
    W1 = nc.declare_dram_parameter("w1", [128, 32, D], MMDT, isOutput=False)
    W2 = nc.declare_dram_parameter("w2", [128, 32, D], MMDT, isOutput=False)
    BQ = nc.declare_dram_parameter("bq", [D], F32, isOutput=False)
    BK = nc.declare_dram_parameter("bk", [D], F32, isOutput=False)
    BV = nc.declare_dram_parameter("bv", [D], F32, isOutput=False)
    BO = nc.declare_dram_parameter("bo", [D], MMDT, isOutput=False)
    B1 = nc.declare_dram_parameter("b1", [DFF], F32, isOutput=False)
    B2 = nc.declare_dram_parameter("b2", [D], MMDT, isOutput=False)
    Y = nc.declare_dram_parameter("y", [QT, D], F32, isOutput=True)

    with TileContext(nc) as tc:
        with (
            tc.tile_pool(name="const", bufs=1) as cpool,
            tc.tile_pool(name="accp", bufs=1) as accp,
        ):
            ident = cpool.tile([128, 128], F32)
            make_identity(nc, ident)
            identb = cpool.tile([128, 128], MMDT)
            make_identity(nc, identb)
            eps = cpool.tile([128, 1], F32)
            nc.vector.memset(eps, EPS)
            onesb = cpool.tile([128, 128], MMDT)  # all-ones, bf16
            nc.vector.memset(onesb, 1.0)
            bqT = cpool.tile([128, 8], F32)
            nc.sync.dma_start(out=bqT, in_=BQ[:].rearrange("(t p) -> p t", p=128))
            bkT = cpool.tile([128, 8], F32)
            nc.sync.dma_start(out=bkT, in_=BK[:].rearrange("(t p) -> p t", p=128))
            b1T = cpool.tile([128, 32], F32)
            nc.sync.dma_start(out=b1T, in_=B1[:].rearrange("(t p) -> p t", p=128))
            bv_bc = cpool.tile([128, D], F32)
            nc.sync.dma_start(out=bv_bc, in_=BV[:].partition_broadcast(128))
            bo_row = cpool.tile([1, D], MMDT)
            nc.sync.dma_start(out=bo_row, in_=BO[:].rearrange("(o n) -> o n", o=1))
            b2_row = cpool.tile([1, D], MMDT)
            nc.sync.dma_start(out=b2_row, in_=B2[:].rearrange("(o n) -> o n", o=1))
            # K/V/O weights cached in SBUF; DMAs issued after the LN-q
            # chain so the critical path is not stuck behind 6MB of weights
            wk_sb = cpool.tile([128, 8, D], MMDT)
            wv_sb = cpool.tile([128, 8, D], MMDT)
            wo_sb = cpool.tile([128, 8, D], MMDT)

            acc = accp.tile([65, 16, 512], MMDT)  # unnormalized attn^T + denom row
            attn128 = accp.tile([128, 8, 512], MMDT)  # normalized attn^T
            x2 = accp.tile([128, 4, D], F32)  # post-attention residual stream
            xq_sb = accp.tile([128, 4, D], F32)

            # ---- projections + attention, streamed + software-pipelined ----
            with (
                tc.tile_pool(name="qp", bufs=1) as qp,
                tc.tile_pool(name="lnp", bufs=2) as lnp,
                tc.tile_pool(name="hTp", bufs=2) as hTp,
                tc.tile_pool(name="ktp", bufs=2) as ktp,
                tc.tile_pool(name="vp", bufs=2) as vp,
                tc.tile_pool(name="wsm", bufs=3) as wsm,
                tc.tile_pool(name="bcp", bufs=2) as bcp,
                tc.tile_pool(name="pp", bufs=4) as ppl,
                tc.tile_pool(name="nrm", bufs=2) as nrm,
                tc.tile_pool(name="psK", bufs=2, space="PSUM") as psK,
                tc.tile_pool(name="psS", bufs=2, space="PSUM") as psS,
                tc.tile_pool(name="psA", bufs=2, space="PSUM") as psA,
            ):
                Q_sb = qp.tile([128, 8, 512], MMDT)  # Q^T [hd, q]

                # LN of the core's own 512 query tokens + Q projection
                hqT = hTp.tile([128, 8, 512], MMDT, tag="hT")
                LNPipe(nc, lnp, psK, psS, bcp, ident, onesb, eps,
                       XQ, XQT, 0, hqT).all()
                nc.sync.dma_start(out=wk_sb, in_=WK[:].rearrange("(t p) n -> p t n", p=128))
                nc.sync.dma_start(out=wv_sb, in_=WV[:].rearrange("(t p) n -> p t n", p=128))
                for ht in range(8):
                    wcol = wsm.tile([128, D], MMDT, tag="w")
                    nc.sync.dma_start(out=wcol, in_=WQ[:, ht, :])
                    psq = psK.tile([128, 512], F32, tag="psK")
                    for dt in range(8):
                        nc.tensor.matmul(
                            psq, wcol[:, dt * 128:(dt + 1) * 128], hqT[:, dt, :],
                            start=(dt == 0), stop=(dt == 7),
                        )
                    nc.vector.tensor_scalar_add(Q_sb[:, ht, :], psq, bqT[:, ht:ht + 1])

                # per-chunk state
                hT_t = [None] * NCHUNK
                KT_t = [None] * NCHUNK
                V_t = [None] * NCHUNK

                def proj_group(kc, g):
                    """Projection group g (0-7: K head-cols, 8-15: V (hc,st))."""
                    hT, KT, V = hT_t[kc], KT_t[kc], V_t[kc]
                    if g < 8:
                        ht = g
                        psk = psK.tile([128, 512], F32, tag="psK")
                        for dt in range(8):
                            nc.tensor.matmul(
                                psk, wk_sb[:, dt, ht * 128:(ht + 1) * 128], hT[:, dt, :],
                                start=(dt == 0), stop=(dt == 7),
                            )
                        nc.vector.tensor_scalar_add(KT[:, ht, :], psk, bkT[:, ht:ht + 1])
                    else:
                        hc, st = (g - 8) // 4, (g - 8) % 4
                        psv = psK.tile([128, 512], F32, tag="psK")
                        for dt in range(8):
                            nc.tensor.matmul(
                                psv,
                                hT[:, dt, st * 128:(st + 1) * 128],
                                wv_sb[:, dt, hc * 512:(hc + 1) * 512],
                                start=(dt == 0),
                                stop=(dt == 7),
                            )
                        nc.vector.tensor_add(
                            V[:, st, hc * 8:(hc + 1) * 8, 0:64],
                            psv.rearrange("p (h d) -> p h d", h=8),
                            bv_bc[:, hc * 512:(hc + 1) * 512].rearrange(
                                "p (h d) -> p h d", h=8
                            ),
                        )

                def new_chunk(kc):
                    hT_t[kc] = hTp.tile([128, 8, 512], MMDT, tag="hT", name=f"hT{kc}")
                    KT_t[kc] = ktp.tile([128, 8, 512], MMDT, tag="KT", name=f"KT{kc}")
                    V_t[kc] = vp.tile([128, 4, 16, 65], MMDT, tag="V", name=f"V{kc}")
                    nc.vector.memset(V_t[kc][:, :, :, 64:65], 1.0)
                    return LNPipe(nc, lnp, psK, psS, bcp, ident, onesb, eps,
                                  XB, XBT, kc * 512, hT_t[kc])

                def scores_exp(kc, h):
                    ko = (h % 2) * 64
                    kj = h // 2
                    KT = KT_t[kc]
                    p_tiles = []
                    for half in range(2):
                        pss = psS.tile([128, 2, 512], F32, tag="psS")
                        for j in range(2):
                            kt = half * 2 + j
                            nc.tensor.matmul(
                                pss[:, j, :],
                                KT[ko:ko + 64, kj, kt * 128:(kt + 1) * 128],
                                Q_sb[ko:ko + 64, kj, :],
                                start=True,
                                stop=True,
                            )
                        P = ppl.tile([128, 2, 512], MMDT, tag="P")
                        nc.scalar.activation(P, pss, AF.Exp, scale=0.125)
                        p_tiles.append(P)
                    return p_tiles

                def av_acc(kc, h, p_tiles):
                    psa = psA.tile([65, 512], F32, tag="psA")
                    V = V_t[kc]
                    for kt in range(4):
                        nc.tensor.matmul(
                            psa, V[:, kt, h, :], p_tiles[kt // 2][:, kt % 2, :],
                            start=(kt == 0), stop=(kt == 3),
                        )
                    if kc == 0:
                        with nc.allow_low_precision(reason="attn accum fits bf16"):
                            nc.vector.tensor_copy(acc[:, h, :], psa)
                    else:
                        with nc.allow_low_precision(reason="attn accum fits bf16"):
                            nc.vector.tensor_add(acc[:, h, :], acc[:, h, :], psa)

                def normalize(h):
                    rr = nrm.tile([1, 512], MMDT, tag="rr")
                    with nc.allow_low_precision(reason="softmax denom fits bf16"):
                        nc.vector.reciprocal(rr, acc[64:65, h, :])
                    rbt = nrm.tile([64, 512], MMDT, tag="rbt")
                    nc.gpsimd.partition_broadcast(rbt, rr)
                    ko = (h % 2) * 64
                    nc.vector.tensor_mul(
                        attn128[ko:ko + 64, h // 2, :], acc[0:64, h, :], rbt
                    )

                # chunk 0: LN + projections up front
                ln = new_chunk(0)
                ln.all()
                nc.sync.dma_start(out=wo_sb, in_=WO[:])
                nc.sync.dma_start(out=xq_sb, in_=XQ[:].rearrange("(t p) n -> p t n", p=128))
                for g in range(16):
                    proj_group(0, g)

                for kc in range(NCHUNK):
                    last = kc == NCHUNK - 1
                    if not last:
                        nln = new_chunk(kc + 1)
                    p_cur = scores_exp(kc, 0)
                    for h in range(H):
                        p_next = scores_exp(kc, h + 1) if h + 1 < H else None
                        if not last:
                            # stage the next chunk's LN so its DVE/DMA work
                            # overlaps this chunk's exp-bound stretches
                            if h == 0:
                                nln.stats()
                            elif h == 4:
                                nln.rows()
                            elif h == 5:
                                nln.apply()
                            elif h >= 8:  # 2 projection groups per head
                                proj_group(kc + 1, 2 * (h - 8))
                                proj_group(kc + 1, 2 * (h - 8) + 1)
                        av_acc(kc, h, p_cur)
                        if last:
                            normalize(h)
                        p_cur = p_next

            # ---- out-projection + residual ----
            with (
                tc.tile_pool(name="psO", bufs=8, space="PSUM") as psO,
            ):
                po = [psO.tile([128, 512], F32, tag="psO", name=f"po{i}") for i in range(8)]
                for j in range(8):
                    wot = wo_sb[:, j, :]
                    for c in range(2):
                        for qt in range(4):
                            nc.tensor.matmul(
                                po[c * 4 + qt], attn128[:, j, qt * 128:(qt + 1) * 128],
                                wot[:, c * 512:(c + 1) * 512],
                                start=(j == 0), stop=False,
                            )
                for qt in range(4):
                    for c in range(2):
                        # fold bo in via rank-1 matmul, then single residual add
                        nc.tensor.matmul(
                            po[c * 4 + qt], onesb[0:1, 0:128],
                            bo_row[:, c * 512:(c + 1) * 512],
                            start=False, stop=True,
                        )
                        nc.vector.tensor_add(
                            x2[:, qt, c * 512:(c + 1) * 512],
                            po[c * 4 + qt],
                            xq_sb[:, qt, c * 512:(c + 1) * 512],
                        )

            # ---- LN2 + MLP + residual ----
            with (
                tc.tile_pool(name="lnp2", bufs=2) as lnp2,
                tc.tile_pool(name="h2p", bufs=1) as h2p,
                tc.tile_pool(name="gp", bufs=1) as gp,
                tc.tile_pool(name="wfp", bufs=6) as wfp,
                tc.tile_pool(name="w2p", bufs=6) as w2p,
                tc.tile_pool(name="yp", bufs=2) as yp,
            ):
                h2T = h2p.tile([128, 8, 512], MMDT)
                G = gp.tile([128, 32, 512], MMDT)
                with (
                    tc.tile_pool(name="psT2", bufs=2, space="PSUM") as psT2,
                    tc.tile_pool(name="psF", bufs=4, space="PSUM") as psF,
                ):
                    # LN2 on the vector engine + PE transposes
                    for st in range(4):
                        xt = x2[:, st, :]
                        stats = lnp2.tile([128, 2, 6], F32, tag="ln_st")
                        nc.vector.bn_stats(stats[:, 0, :], xt[:, 0:512])
                        nc.vector.bn_stats(stats[:, 1, :], xt[:, 512:1024])
                        mv = lnp2.tile([128, 2], F32, tag="ln_mv")
                        nc.vector.bn_aggr(mv, stats)
                        sd2 = lnp2.tile([128, 1], F32, tag="ln_sd")
                        nc.scalar.activation(sd2, mv[:, 1:2], AF.Sqrt, bias=eps[:, 0:1])
                        rstd = lnp2.tile([128, 1], F32, tag="ln_rs")
                        nc.vector.reciprocal(rstd, sd2)
                        h = lnp2.tile([128, D], MMDT, tag="ln_h")
                        nc.vector.tensor_scalar(h, xt, mv[:, 0:1], rstd[:, 0:1],
                                                ALU.subtract, ALU.mult)
                        for dt in range(8):
                            pst = psT2.tile([128, 128], MMDT, tag="tp")
                            nc.tensor.transpose(pst, h[:, dt * 128:(dt + 1) * 128], identb)
                            nc.vector.tensor_copy(h2T[:, dt, st * 128:(st + 1) * 128], pst)

                    # MLP1: gelu(h2 @ w1 + b1), transposed output [dff, q]
                    for ft in range(32):
                        w1c = wfp.tile([128, D], MMDT, tag="w1")
                        nc.sync.dma_start(out=w1c, in_=W1[:, ft, :])
                        psf = psF.tile([128, 512], F32, tag="psF")
                        for dt in range(8):
                            nc.tensor.matmul(
                                psf, w1c[:, dt * 128:(dt + 1) * 128], h2T[:, dt, :],
                                start=(dt == 0), stop=(dt == 7),
                            )
                        nc.scalar.activation(
                            G[:, ft, :], psf, AF.Gelu, bias=b1T[:, ft:ft + 1]
                        )

                # MLP2: y = G^T @ w2 + b2 + x2
                with tc.tile_pool(name="psY", bufs=8, space="PSUM") as psY:
                    py = [psY.tile([128, 512], F32, tag="psY", name=f"py{i}") for i in range(8)]
                    for ft in range(32):
                        w2t = w2p.tile([128, D], MMDT, tag="w2")
                        nc.sync.dma_start(out=w2t, in_=W2[:, ft, :])
                        for c in range(2):
                            for qt in range(4):
                                nc.tensor.matmul(
                                    py[c * 4 + qt], G[:, ft, qt * 128:(qt + 1) * 128],
                                    w2t[:, c * 512:(c + 1) * 512],
                                    start=(ft == 0), stop=False,
                                )
                    for c in range(2):
                        for qt in range(4):
                            nc.tensor.matmul(
                                py[c * 4 + qt], onesb[0:1, 0:128],
                                b2_row[:, c * 512:(c + 1) * 512],
                                start=False, stop=True,
                            )
                            yt = yp.tile([128, 512], F32, tag="yt2")
                            nc.vector.tensor_add(
                                yt, py[c * 4 + qt], x2[:, qt, c * 512:(c + 1) * 512]
                            )
                            nc.sync.dma_start(
                                out=Y[qt * 128:(qt + 1) * 128, c * 512:(c + 1) * 512],
                                in_=yt,
                            )

    nc.compile()
    return nc


_NC = None


def _get_nc():
    global _NC
    if _NC is None:
        _NC = _build()
    return _NC


def _make_in_maps(x, ln1_g, ln1_b, wq, bq, wk, bk, wv, bv, wo, bo,
                  w1, b1, w2, b2, ln2_g, ln2_b):
    f32 = lambda a: np.ascontiguousarray(np.asarray(a, dtype=np.float32))
    bf = lambda a: np.ascontiguousarray(np.asarray(a, dtype=np.float32).astype(BF))
    x = f32(x)
    ln1_g, ln1_b = f32(ln1_g), f32(ln1_b)
    ln2_g, ln2_b = f32(ln2_g), f32(ln2_b)
    wq, wk, wv, wo = f32(wq), f32(wk), f32(wv), f32(wo)
    w1, w2 = f32(w1), f32(w2)
    bq, bk, bv, bo, b1, b2 = f32(bq), f32(bk), f32(bv), f32(bo), f32(b1), f32(b2)

    # Fold LayerNorm affine params into the following projections (exact).
    # Weight layouts are pre-rearranged so every on-chip DMA reads dense
    # 2KB-per-partition lines:
    #   wq/wo: [p, blk, t*128+j] = w[t*128+p, blk*128+j]
    #   w1:    [p, ft, t*128+j] = w1[t*128+p, ft*128+j]
    #   w2:    [p, ft, n]       = w2[ft*128+p, n]
    def colmajor(w, nblk):  # [1024, nblk*128] -> [128, nblk, 1024]
        return np.ascontiguousarray(
            w.reshape(8, 128, nblk, 128).transpose(1, 2, 0, 3).reshape(128, nblk, 1024))
    def rowmajor(w, nblk):  # [nblk*128, 1024] -> [128, nblk, 1024]
        return np.ascontiguousarray(
            w.reshape(nblk, 128, 1024).transpose(1, 0, 2))
    common = {
        "wq": bf(colmajor(ln1_g[:, None] * wq, 8)),
        "wk": bf(ln1_g[:, None] * wk),
        "wv": bf(ln1_g[:, None] * wv),
        "wo": bf(rowmajor(wo, 8)),
        "w1": bf(colmajor(ln2_g[:, None] * w1, 32)),
        "w2": bf(rowmajor(w2, 32)),
        "bq": f32(bq + ln1_b @ wq),
        "bk": f32(bk + ln1_b @ wk),
        "bv": f32(bv + ln1_b @ wv),
        "bo": bf(bo),
        "b1": f32(b1 + ln2_b @ w1),
        "b2": bf(b2),
    }
    in_maps = []
    for c in range(NCORES):
        b = c // 4
        qoff = (c % 4) * QT
        m = dict(common)
        m["xb"] = bf(x[b])
        m["xq"] = np.ascontiguousarray(x[b, qoff:qoff + QT])
        m["xbt"] = bf(x[b].T)
        m["xqt"] = bf(x[b, qoff:qoff + QT].T)
        in_maps.append(m)
    return in_maps


def kernel(x, ln1_g, ln1_b, wq, bq, wk, bk, wv, bv, wo, bo, w1, b1, w2, b2, ln2_g, ln2_b):
    in_maps = _make_in_maps(x, ln1_g, ln1_b, wq, bq, wk, bk, wv, bv, wo, bo,
                            w1, b1, w2, b2, ln2_g, ln2_b)
    nc = _get_nc()
    res = run_bass_kernel_spmd(nc, in_maps, core_ids=list(range(NCORES)))

    y = np.empty((B, S, D), dtype=np.float32)
    for c in range(NCORES):
        b = c // 4
        qoff = (c % 4) * QT
        y[b, qoff:qoff + QT] = res.results[c]["y"]
    return y
